# revision 13
# baseline (speedup 1.0000x reference)
"""Causal dot-product attention (low-rank V) on 8 Trainium2 NeuronCores.

Problem: inputs [B=4, N=4096, E=1024], Wq/Wk/Wvdown [E, D=256], Wvup [D, E].
    Q = x Wq; K = x Wk; S = Q K^T / sqrt(D) (causal); A = softmax(S)
    V = x Wvdown Wvup; out = A V

Sharding: core = (batch, key-parity). Each of the 4 batches is handled by a
pair of cores; core parity c owns the interleaved global key blocks {2j+c}
(128 rows each), which balances the causal work exactly. Each core computes
full Q for its batch, K/Vd for its key half, and produces the *unnormalized*
attention output O_unnorm plus softmax row-sums. The host combines:
out = (O_even + O_odd) / (s_even + s_odd).

The kernel program is parity-uniform: the host swaps adjacent 128-row block
pairs of x for odd cores (and hands matching diagonal masks), so every core's
keys sit at the EVEN 128-column blocks of its query stream. K and Vd then
project strided SBUF views of the already-loaded x tiles -- no separate
key-activation DMA at all. The host un-swaps the odd cores' output rows.

Low-rank V is exploited on-device: O = A V = (A (x Wvd)) Wvup, so the wide
(E=1024) contraction happens once per query row against the rank-D attention
result instead of once per (query, key-block) pair. Scores are computed
transposed, ST[k, q] = K Q^T, so the exp'd tile P[k, q] is directly the
moving operand of the OT' = Vd^T P accumulation (no on-device transposes).

All activations/weights stream as bf16 (error budget ~0.5% vs the 2e-2
gate): halves HBM traffic and enables FWL fast weight loads on the PE.
A short dummy-matmul chain at kernel start keeps the PE busy through the
initial DMA wait so the HAM clock gate reaches 2.4 GHz before real work.
The scores stage is interleaved with the previous chunk's Wvup out-stage so
the ACT engine's exp stream (690ns/tile vs the PE's 426ns/tile) never
backs the PE up against the 3-deep score-PSUM ring.
"""

import sys

sys.path.insert(0, "/opt/trn_rl_repo")

import numpy as np

import concourse.bacc as bacc
import concourse.mybir as mybir
import concourse.tile as tile

F32 = mybir.dt.float32
F32R = mybir.dt.float32r
BF16 = mybir.dt.bfloat16

B, N, E, D = 4, 4096, 1024, 256
NCORES = 8
KLOC = N // 2  # local keys per core
NKB = KLOC // 128  # 16 local key blocks
NQC = N // 512  # 8 query chunks of 512
SCALE = 1.0 / np.sqrt(np.float32(D))  # 1/16

_CACHE = {}


def _key_view(x_ap):
    """Strided view of a [128, 512] x-chunk AP selecting its two even
    128-column blocks (the key columns) as a [128, 2, 128] AP."""
    return x_ap.rearrange("p (g two q) -> p g two q", g=2, two=2, q=128)[:, :, 0, :]


def _build_nc(reps=1):
    nc = bacc.Bacc("TRN2", target_bir_lowering=False)

    # Weights arrive host-packed in the SBUF e-chunk layout ([128, 8, 256]
    # flattened: row p holds [c, d] for e-row c*128+p) so each is one fully
    # contiguous DMA.
    xT = nc.dram_tensor("xT", [E, N], BF16, kind="ExternalInput")
    wq = nc.dram_tensor("wq", [128, E * D // 128], BF16, kind="ExternalInput")
    wk = nc.dram_tensor("wk", [128, E * D // 128], BF16, kind="ExternalInput")
    wvd = nc.dram_tensor("wvd", [128, E * D // 128], BF16, kind="ExternalInput")
    wvu = nc.dram_tensor("wvu", [128, D * E // 128], BF16, kind="ExternalInput")
    mk = nc.dram_tensor("mk", [128, 1024], BF16, kind="ExternalInput")

    o = nc.dram_tensor("o", [N, E], BF16, kind="ExternalOutput")
    ssum = nc.dram_tensor("ssum", [1, N], F32, kind="ExternalOutput")

    with tile.TileContext(nc) as tc:
      for _rep in range(reps):
        with (
            tc.tile_pool(name=f"res{_rep}", bufs=1) as res,
            tc.tile_pool(name=f"consts{_rep}", bufs=1) as consts,
            tc.tile_pool(name=f"wpool{_rep}", bufs=1) as wp,
            tc.tile_pool(name=f"xstream{_rep}", bufs=3) as xs,
            tc.tile_pool(name=f"accpool{_rep}", bufs=2) as accpool,
            tc.tile_pool(name=f"ppool{_rep}", bufs=2) as ppool,
            tc.tile_pool(name=f"stage{_rep}", bufs=3) as stage,
            tc.tile_pool(name=f"ps{_rep}", bufs=4, space="PSUM") as pps,
            tc.tile_pool(name=f"ps_ot{_rep}", bufs=2, space="PSUM") as ps_ot,
            tc.tile_pool(name=f"ps_o{_rep}", bufs=2, space="PSUM") as ps_o,
        ):
            # PE warm-up: a dependency-free accumulation chain issued ahead
            # of everything keeps the PE busy through the initial DMA wait
            # (~7us: weights + first x pieces at ~130 GB/s/ring), so the HAM
            # clock gate un-throttles before the first real matmul and the
            # activity window stays hot through the DMA-paced first chunk.
            wrm = consts.tile([128, 128], BF16, tag="wrm")
            nc.gpsimd.memset(wrm, 0.0)
            scr = consts.tile([1, 1], F32, tag="scr")
            wps = pps.tile([128, 512], F32, tag="ps")
            NWARM = 28
            for j in range(NWARM):
                nc.tensor.matmul(
                    wps[:, :128], lhsT=wrm, rhs=wrm,
                    start=(j == 0), stop=(j == NWARM - 1),
                )
            nc.vector.tensor_copy(scr, wps[:1, :1])

            # Resident results of the projection phase.
            qt = [res.tile([128, N], BF16, tag=f"qt{d}", name=f"qt{d}") for d in range(2)]
            kt = [res.tile([128, KLOC], BF16, tag=f"kt{d}", name=f"kt{d}") for d in range(2)]
            # Vd tiles grouped per chunk (2 key blocks each): block kb lives
            # at vd[kb // 2][:, kb % 2, :].
            vd = [
                res.tile([128, 2, D], BF16, tag=f"vd{i}", name=f"vd{i}")
                for i in range(NQC)
            ]
            srow = res.tile([1, N], F32, tag="srow")

            ones_f = consts.tile([128, 1], F32, tag="ones_f")
            nc.vector.memset(ones_f, 1.0)
            ones_r = consts.tile([128, 1], F32R, tag="ones_r")
            nc.vector.tensor_copy(ones_r, ones_f)
            mk_t = consts.tile([128, 1024], BF16, tag="mk")
            mask_a = mk_t[:, :512]
            mask_b = mk_t[:, 512:]
            wvu_t = consts.tile([128, 2, E], BF16, tag="wvu")

            # One contiguous DMA per weight matrix; wk split across both
            # rings so the first KT chain can start as soon as possible.
            wkt = wp.tile([128, 8, D], BF16, tag="wk", name="wkt")
            wvdt = wp.tile([128, 8, D], BF16, tag="wvd", name="wvdt")
            wqt = wp.tile([128, 8, D], BF16, tag="wq", name="wqt")
            nc.sync.dma_start(
                out=wkt[:, :4, :],
                in_=wk[:, : 4 * D].rearrange("p (c d) -> p c d", c=4),
            )
            nc.scalar.dma_start(
                out=wkt[:, 4:, :],
                in_=wk[:, 4 * D :].rearrange("p (c d) -> p c d", c=4),
            )

            # x streams in 1024-query loads (2KB HBM lines), two half-E
            # tiles per load, one per HWDGE ring; each load covers two
            # 512-query processing sub-chunks.
            xtiles = {}

            def load_x(L):
                xa = xs.tile([128, 4, 2 * 512], BF16, tag="xa", bufs=2, name="xa")
                xb = xs.tile([128, 4, 2 * 512], BF16, tag="xb", bufs=2, name="xb")
                xtiles[L] = (xa, xb)
                for t, base, eng in ((xa, 0, nc.sync), (xb, 512, nc.scalar)):
                    if L == 0:
                        # four pieces so the first KT chains start on the
                        # first 256KB
                        for qh in range(2):
                            for ch in range(2):
                                eng.dma_start(
                                    out=t[
                                        :, 2 * ch : 2 * ch + 2, qh * 512 : qh * 512 + 512
                                    ],
                                    in_=xT[
                                        base + ch * 256 : base + ch * 256 + 256,
                                        qh * 512 : qh * 512 + 512,
                                    ].rearrange("(c p) q -> p c q", p=128),
                                )
                            if qh == 0 and base == 0:
                                # weights ordered by first PE use: Vd -> Q
                                nc.sync.dma_start(
                                    out=wvdt,
                                    in_=wvd[:, :].rearrange("p (c d) -> p c d", c=8),
                                )
                            if qh == 0 and base == 512:
                                nc.scalar.dma_start(
                                    out=wqt,
                                    in_=wq[:, :].rearrange("p (c d) -> p c d", c=8),
                                )
                                nc.scalar.dma_start(out=mk_t, in_=mk[:, :])
                    else:
                        eng.dma_start(
                            out=t,
                            in_=xT[
                                base : base + 512, L * 1024 : (L + 1) * 1024
                            ].rearrange("(c p) q -> p c q", p=128),
                        )
                if L == 1:
                    nc.scalar.dma_start(
                        out=wvu_t,
                        in_=wvu[:, :].rearrange("p (c d) -> p c d", c=2),
                    )

            def proj_sub(i):
                """Projections for 512-query sub-chunk i: QT for its
                queries, KT/Vd for the two key blocks embedded in it (KT is
                emitted once per load at N=512, except the piece-split
                first load)."""
                L, s = i // 2, i % 2
                xa, xb = xtiles[L]

                def xch(c):
                    t = xa if c < 4 else xb
                    return t[:, c % 4, s * 512 : (s + 1) * 512]

                # KT[d, keys] from the even column blocks.
                if L == 0 or s == 0:
                    for d in range(2):
                        ps = pps.tile([128, 512], F32, tag="ps")
                        dsl = slice(d * 128, (d + 1) * 128)
                        if L == 0:
                            nkeys, ksl = 256, slice(i * 256, (i + 1) * 256)
                            kview = [_key_view(xch(c)) for c in range(8)]
                        else:
                            nkeys, ksl = 512, slice(L * 512, (L + 1) * 512)
                            kview = [
                                (xa if c < 4 else xb)[:, c % 4, :].rearrange(
                                    "p (g two q) -> p g two q", g=4, two=2, q=128
                                )[:, :, 0, :]
                                for c in range(8)
                            ]
                        for c in range(8):
                            nc.tensor.matmul(
                                ps[:, :nkeys],
                                lhsT=(wkt[:, c, dsl]),
                                rhs=(kview[c]),
                                start=(c == 0),
                                stop=(c == 7),
                            )
                        nc.vector.tensor_copy(kt[d][:, ksl], ps[:, :nkeys])
                # Vd[k, d] (partition = keys) for key blocks 2i, 2i+1, which
                # sit at sub-chunk columns 0:128 and 256:384.
                for h in range(2):
                    pvp = pps.tile([128, 512], F32, tag="ps")
                    csl = slice(h * 256, h * 256 + 128)
                    for c in range(8):
                        nc.tensor.matmul(
                            pvp[:, :D],
                            lhsT=(xch(c)[:, csl]),
                            rhs=(wvdt[:, c, :]),
                            start=(c == 0),
                            stop=(c == 7),
                        )
                    nc.vector.tensor_copy(vd[i][:, h, :], pvp[:, :D])

                for d in range(2):
                    ps = pps.tile([128, 512], F32, tag="ps")
                    dsl = slice(d * 128, (d + 1) * 128)
                    for c in range(8):
                        nc.tensor.matmul(
                            ps,
                            lhsT=(wqt[:, c, dsl]),
                            rhs=(xch(c)),
                            start=(c == 0),
                            stop=(c == 7),
                        )
                    nc.vector.tensor_copy(qt[d][:, i * 512 : (i + 1) * 512], ps)

            def stage_attn(qc, ppool, out_emitters=()):
                """Fused scores + rank-D reduction for query chunk qc.

                Score block kb: ST = K Q^T matmuls + exp + diagonal mask ->
                P tile; the OT'[d, q] += Vd[k, d]^T P[k, q] accumulation for
                block kb trails LAG blocks behind in the PE stream (both
                d-half chains interleaved per block), so each P tile's last
                use follows its exp closely: the P pool needs only 8 ring
                slots, and the PE's per-block cost (3 matmul pairs, ~1.3us)
                exceeds the ACT exp cost (~0.7us), so the exp stream never
                backs the PE up against the 3-deep score-PSUM ring.

                out_emitters: closures emitting the chunk-(qc-1) Wvup
                out-stage, interleaved at blocks 3 and 7 to spread the
                staging-copy and output-DMA load."""
                out_emitters = list(out_emitters)
                nb = 2 * qc + 2
                LAG = 4
                acc = accpool.tile([128, 512], F32, tag="acc")
                otp = [
                    ps_ot.tile([128, 512], F32, tag="otp", name=f"otp{_d}")
                    for _d in range(2)
                ]
                pts = []
                for step in range(nb + LAG):
                  if step < nb:
                    kb = step
                    ksl = slice(kb * 128, (kb + 1) * 128)
                    # The final diagonal block's first 256 query columns are
                    # fully masked for both parities: compute only the upper
                    # half and zero the rest.
                    half = kb == nb - 1
                    csl = slice(256, 512) if half else slice(0, 512)
                    st = pps.tile([128, 512], F32, tag="ps")
                    for d in range(2):
                        nc.tensor.matmul(
                            st[:, csl],
                            lhsT=(kt[d][:, ksl]),
                            rhs=(qt[d][:, qc * 512 + csl.start : qc * 512 + csl.stop]),
                            start=(d == 0),
                            stop=(d == 1),
                        )
                    pt = ppool.tile(
                        [128, 512], BF16, tag=f"p{kb % 4}", bufs=2, name=f"p{kb % 4}"
                    )
                    if half:
                        nc.vector.memset(pt[:, :256], 0.0)
                    nc.scalar.activation(
                        pt[:, csl], st[:, csl],
                        mybir.ActivationFunctionType.Exp, scale=float(SCALE),
                    )
                    # The last two blocks straddle the causal diagonal.
                    if kb == nb - 2:
                        nc.vector.tensor_mul(pt, pt, mask_a)
                    elif kb == nb - 1:
                        nc.vector.tensor_mul(
                            pt[:, 256:], pt[:, 256:], mask_b[:, 256:]
                        )
                    pts.append(pt)
                    # Softmax-denominator accumulation, column-split across
                    # the Pool and DVE engines to keep pace with the exps.
                    if kb == 0:
                        nc.gpsimd.tensor_copy(acc[:, :256], pt[:, :256])
                        nc.vector.tensor_copy(acc[:, 256:], pt[:, 256:])
                    else:
                        nc.gpsimd.tensor_add(acc[:, :256], acc[:, :256], pt[:, :256])
                        nc.vector.tensor_add(acc[:, 256:], acc[:, 256:], pt[:, 256:])
                    if kb in (3, 7) and out_emitters:
                        out_emitters.pop(0)()
                  if step >= LAG:
                    kb = step - LAG
                    for dh in range(2):
                        nc.tensor.matmul(
                            otp[dh],
                            lhsT=(vd[kb // 2][:, kb % 2, dh * 128 : (dh + 1) * 128]),
                            rhs=(pts[kb]),
                            start=(kb == 0),
                            stop=(kb == nb - 1),
                        )
                for em in out_emitters:
                    em()
                return otp, acc

            def stage_reduce(qc, otp, acc, stage):
                """R: stage the rank-D attention result out of PSUM, then
                the softmax denominators."""
                ots = []
                for dh in range(2):
                    ot = stage.tile([128, 512], BF16, tag=f"ot{dh}", bufs=2)
                    # Split the PSUM->SBUF copy so the first Wvup matmuls can
                    # start after half the copy has landed.
                    if qc == NQC - 1:
                        # final chunk: ACT still holds the exp backlog and
                        # nothing follows to hide it; DVE is idle here
                        nc.vector.tensor_copy(ot[:, :256], otp[dh][:, :256])
                        nc.vector.tensor_copy(ot[:, 256:], otp[dh][:, 256:])
                    else:
                        nc.scalar.activation(
                            ot[:, :256], otp[dh][:, :256],
                            mybir.ActivationFunctionType.Copy,
                        )
                        nc.scalar.activation(
                            ot[:, 256:], otp[dh][:, 256:],
                            mybir.ActivationFunctionType.Copy,
                        )
                    ots.append(ot)
                # sums[1, q] = 1^T acc[k, q] -- a single matmul per chunk; it
                # also covers the ot-copy latency before stage_out's first
                # matmul. The PE wants f32r operands, and f32r matmul inputs
                # must be produced rounded, so DVE stages acc into f32r.
                acc_r = stage.tile([128, 512], F32R, tag="accr", bufs=2)
                nc.vector.tensor_copy(acc_r, acc)
                sums = ps_o.tile([128, 512], F32, tag="ops", name="sums")
                nc.tensor.matmul(
                    sums[:1, :], lhsT=(ones_r), rhs=(acc_r), start=True, stop=True
                )
                nc.vector.tensor_copy(srow[:, qc * 512 : (qc + 1) * 512], sums[:1, :])
                return ots

            def make_out_emitters(qc, ots, stage):
                """W: O[q, e] = OT'[d, q]^T Wvup[d, e]; two emitters, one
                per 256-query half, each ending in one output DMA on the
                scalar HWDGE ring. The final chunk instead DMAs each
                128-query block as soon as it is staged (alternating
                rings) to shorten the completion tail."""
                obt = stage.tile([128, 4, E], BF16, tag="obt", bufs=2)
                last = qc == NQC - 1

                def emit(half):
                    for q4 in (2 * half, 2 * half + 1):
                        q4sl = slice(q4 * 128, (q4 + 1) * 128)
                        for eh in range(2):
                            esl = slice(eh * 512, (eh + 1) * 512)
                            ops = ps_o.tile([128, 512], F32, tag="ops")
                            for dh in range(2):
                                nc.tensor.matmul(
                                    ops,
                                    lhsT=(ots[dh][:, q4sl]),
                                    rhs=(wvu_t[:, dh, esl]),
                                    start=(dh == 0),
                                    stop=(dh == 1),
                                )
                            if eh == 0:
                                nc.vector.tensor_copy(obt[:, q4, esl], ops)
                            else:
                                nc.scalar.activation(
                                    obt[:, q4, esl], ops,
                                    mybir.ActivationFunctionType.Copy,
                                )
                        if last:
                            eng = nc.sync if q4 % 2 == 0 else nc.scalar
                            eng.dma_start(
                                out=o[
                                    (qc * 4 + q4) * 128 : (qc * 4 + q4 + 1) * 128, :
                                ],
                                in_=obt[:, q4, :],
                            )
                    if not last:
                        nc.scalar.dma_start(
                            out=o[
                                (qc * 4 + 2 * half) * 128 : (qc * 4 + 2 * half + 2)
                                * 128,
                                :,
                            ].rearrange("(c p) e -> p c e", p=128),
                            in_=obt[:, 2 * half : 2 * half + 2, :],
                        )

                return [lambda: emit(0), lambda: emit(1)]

            # Merged software pipeline: the projection stream (pipe-bound)
            # interleaves with the attention stages (PE/exp-bound) one chunk
            # behind, so each fills the other's stalls. Order per step --
            # proj(i), scores(i-1) [with out(i-2) interleaved], reduce(i-1).
            ots = None
            for i in range(NQC):
                if i == 0:
                    load_x(0)
                    load_x(1)
                elif i in (1, 3):
                    load_x((i + 3) // 2)
                proj_sub(i)
                if i >= 1:
                    ems = make_out_emitters(i - 2, ots, stage) if i >= 2 else ()
                    otp, acc = stage_attn(i - 1, ppool, ems)
                    ots = stage_reduce(i - 1, otp, acc, stage)
            ems = make_out_emitters(NQC - 2, ots, stage)
            otp, acc = stage_attn(NQC - 1, ppool, ems)
            ots = stage_reduce(NQC - 1, otp, acc, stage)
            nc.sync.dma_start(out=ssum[:, :], in_=srow)
            for em in make_out_emitters(NQC - 1, ots, stage):
                em()
    nc.finalize()
    return nc


def _get_nc():
    if "nc" not in _CACHE:
        _CACHE["nc"] = _build_nc()
    return _CACHE["nc"]


def _host_masks(parity: int):
    """Diagonal-block masks in kernel coordinates. For odd cores the query
    order inside each 256-row pair is swapped (host block-pair permutation),
    so the in-chunk query offset is 128*(block^parity) + within."""
    import ml_dtypes

    yb = np.arange(512) // 128
    ym = np.arange(512) % 128
    q_off = 128 * (yb ^ parity) + ym  # [512] original query offset in chunk
    x = np.arange(128)[:, None]
    mask_a = (q_off[None, :] >= 128 * parity + x).astype(ml_dtypes.bfloat16)
    mask_b = (q_off[None, :] >= 256 + 128 * parity + x).astype(ml_dtypes.bfloat16)
    return np.ascontiguousarray(np.concatenate([mask_a, mask_b], axis=1))


def _swap_pairs(rows):
    """Swap adjacent 128-row block pairs along axis 0."""
    n = rows.shape[0]
    return np.ascontiguousarray(
        rows.reshape(n // 256, 2, 128, *rows.shape[1:])[:, ::-1].reshape(rows.shape)
    )


def _pack_w(w):
    """[C*128, F] -> [128, C*F]: row p holds [c, f] for source row c*128+p
    (the kernel's SBUF e-chunk layout, made host-side so the weight DMA is
    one fully contiguous copy)."""
    import ml_dtypes

    w = np.asarray(w, dtype=np.float32).astype(ml_dtypes.bfloat16)
    c, f = w.shape[0] // 128, w.shape[1]
    return np.ascontiguousarray(
        w.reshape(c, 128, f).transpose(1, 0, 2).reshape(128, c * f)
    )


def _make_in_maps(inputs, Wq, Wk, Wvdown, Wvup):
    import ml_dtypes

    inputs = np.asarray(inputs, dtype=np.float32)
    Wq = _pack_w(Wq)
    Wk = _pack_w(Wk)
    Wvdown = _pack_w(Wvdown)
    Wvup = _pack_w(Wvup)

    in_maps = []
    for core in range(NCORES):
        b, parity = core // 2, core % 2
        xb = inputs[b]  # [N, E]
        if parity:
            xb = _swap_pairs(xb)
        xT = np.ascontiguousarray(xb.T).astype(ml_dtypes.bfloat16)  # [E, N]
        in_maps.append(
            {
                "xT": xT,
                "wq": Wq,
                "wk": Wk,
                "wvd": Wvdown,
                "wvu": Wvup,
                "mk": _host_masks(parity),
            }
        )
    return in_maps


def _combine(results):
    out = np.empty((B, N, E), dtype=np.float32)
    for b in range(B):
        o_e = np.asarray(results[2 * b]["o"], dtype=np.float32)
        o_o = np.asarray(results[2 * b + 1]["o"], dtype=np.float32)
        s_e = np.asarray(results[2 * b]["ssum"], dtype=np.float32).reshape(N)
        s_o = np.asarray(results[2 * b + 1]["ssum"], dtype=np.float32).reshape(N)
        # odd-parity core produced rows in block-pair-swapped order
        o_o = _swap_pairs(o_o)
        s_o = _swap_pairs(s_o)
        out[b] = (o_e + o_o) / (s_e + s_o)[:, None]
    return out


def kernel(inputs, Wq, Wk, Wvdown, Wvup):
    from concourse.bass_utils import run_bass_kernel_spmd

    nc = _get_nc()
    in_maps = _make_in_maps(inputs, Wq, Wk, Wvdown, Wvup)
    res = run_bass_kernel_spmd(nc, in_maps, core_ids=list(range(NCORES)))
    return _combine(res.results)


# revision 14
# speedup vs baseline: 1.0192x; 1.0192x over previous
"""Causal dot-product attention (low-rank V) on 8 Trainium2 NeuronCores.

Problem: inputs [B=4, N=4096, E=1024], Wq/Wk/Wvdown [E, D=256], Wvup [D, E].
    Q = x Wq; K = x Wk; S = Q K^T / sqrt(D) (causal); A = softmax(S)
    V = x Wvdown Wvup; out = A V

Sharding: core = (batch, key-parity). Each of the 4 batches is handled by a
pair of cores; core parity c owns the interleaved global key blocks {2j+c}
(128 rows each), which balances the causal work exactly. Each core computes
full Q for its batch, K/Vd for its key half, and produces the *unnormalized*
attention output O_unnorm plus softmax row-sums. The host combines:
out = (O_even + O_odd) / (s_even + s_odd).

The kernel program is parity-uniform: the host swaps adjacent 128-row block
pairs of x for odd cores (and hands matching diagonal masks), so every core's
keys sit at the EVEN 128-column blocks of its query stream. K and Vd then
project strided SBUF views of the already-loaded x tiles -- no separate
key-activation DMA at all. The host un-swaps the odd cores' output rows.

Low-rank V is exploited on-device: O = A V = (A (x Wvd)) Wvup, so the wide
(E=1024) contraction happens once per query row against the rank-D attention
result instead of once per (query, key-block) pair. Scores are computed
transposed, ST[k, q] = K Q^T, so the exp'd tile P[k, q] is directly the
moving operand of the OT' = Vd^T P accumulation (no on-device transposes).

All activations/weights stream as bf16 (error budget ~0.5% vs the 2e-2
gate): halves HBM traffic and enables FWL fast weight loads on the PE.
A short dummy-matmul chain at kernel start keeps the PE busy through the
initial DMA wait so the HAM clock gate reaches 2.4 GHz before real work.
The scores stage is interleaved with the previous chunk's Wvup out-stage so
the ACT engine's exp stream (690ns/tile vs the PE's 426ns/tile) never
backs the PE up against the 3-deep score-PSUM ring.
"""

import sys

sys.path.insert(0, "/opt/trn_rl_repo")

import numpy as np

import concourse.bacc as bacc
import concourse.mybir as mybir
import concourse.tile as tile

F32 = mybir.dt.float32
F32R = mybir.dt.float32r
BF16 = mybir.dt.bfloat16

B, N, E, D = 4, 4096, 1024, 256
NCORES = 8
KLOC = N // 2  # local keys per core
NKB = KLOC // 128  # 16 local key blocks
NQC = N // 512  # 8 query chunks of 512
SCALE = 1.0 / np.sqrt(np.float32(D))  # 1/16

_CACHE = {}


def _key_view(x_ap):
    """Strided view of a [128, 512] x-chunk AP selecting its two even
    128-column blocks (the key columns) as a [128, 2, 128] AP."""
    return x_ap.rearrange("p (g two q) -> p g two q", g=2, two=2, q=128)[:, :, 0, :]


def _build_nc(reps=1):
    nc = bacc.Bacc("TRN2", target_bir_lowering=False)

    # Weights arrive host-packed in the SBUF e-chunk layout ([128, 8, 256]
    # flattened: row p holds [c, d] for e-row c*128+p) so each is one fully
    # contiguous DMA.
    xT = nc.dram_tensor("xT", [E, N], BF16, kind="ExternalInput")
    wq = nc.dram_tensor("wq", [128, E * D // 128], BF16, kind="ExternalInput")
    wk = nc.dram_tensor("wk", [128, E * D // 128], BF16, kind="ExternalInput")
    wvd = nc.dram_tensor("wvd", [128, E * D // 128], BF16, kind="ExternalInput")
    wvu = nc.dram_tensor("wvu", [128, D * E // 128], BF16, kind="ExternalInput")
    mk = nc.dram_tensor("mk", [128, 1024], BF16, kind="ExternalInput")

    o = nc.dram_tensor("o", [N, E], BF16, kind="ExternalOutput")
    ssum = nc.dram_tensor("ssum", [1, N], F32, kind="ExternalOutput")

    with tile.TileContext(nc) as tc:
      for _rep in range(reps):
        with (
            tc.tile_pool(name=f"res{_rep}", bufs=1) as res,
            tc.tile_pool(name=f"consts{_rep}", bufs=1) as consts,
            tc.tile_pool(name=f"wpool{_rep}", bufs=1) as wp,
            tc.tile_pool(name=f"xstream{_rep}", bufs=3) as xs,
            tc.tile_pool(name=f"accpool{_rep}", bufs=2) as accpool,
            tc.tile_pool(name=f"ppool{_rep}", bufs=2) as ppool,
            tc.tile_pool(name=f"stage{_rep}", bufs=3) as stage,
            tc.tile_pool(name=f"ps{_rep}", bufs=4, space="PSUM") as pps,
            tc.tile_pool(name=f"ps_ot{_rep}", bufs=2, space="PSUM") as ps_ot,
            tc.tile_pool(name=f"ps_o{_rep}", bufs=2, space="PSUM") as ps_o,
        ):
            # PE warm-up: a dependency-free accumulation chain issued ahead
            # of everything keeps the PE busy through the initial DMA wait
            # (~7us: weights + first x pieces at ~130 GB/s/ring), so the HAM
            # clock gate un-throttles before the first real matmul and the
            # activity window stays hot through the DMA-paced first chunk.
            wrm = consts.tile([128, 128], BF16, tag="wrm")
            nc.gpsimd.memset(wrm, 0.0)
            scr = consts.tile([1, 1], F32, tag="scr")
            wps = pps.tile([128, 512], F32, tag="ps")
            NWARM = 48
            for j in range(NWARM):
                nc.tensor.matmul(
                    wps[:, :128], lhsT=wrm, rhs=wrm,
                    start=(j == 0), stop=(j == NWARM - 1),
                )
            nc.vector.tensor_copy(scr, wps[:1, :1])

            # Resident results of the projection phase.
            qt = [res.tile([128, N], BF16, tag=f"qt{d}", name=f"qt{d}") for d in range(2)]
            kt = [res.tile([128, KLOC], BF16, tag=f"kt{d}", name=f"kt{d}") for d in range(2)]
            # Vd tiles grouped per chunk (2 key blocks each): block kb lives
            # at vd[kb // 2][:, kb % 2, :].
            vd = [
                res.tile([128, 2, D], BF16, tag=f"vd{i}", name=f"vd{i}")
                for i in range(NQC)
            ]
            srow = res.tile([1, N], F32, tag="srow")

            ones_f = consts.tile([128, 1], F32, tag="ones_f")
            nc.vector.memset(ones_f, 1.0)
            ones_r = consts.tile([128, 1], F32R, tag="ones_r")
            nc.vector.tensor_copy(ones_r, ones_f)
            mk_t = consts.tile([128, 1024], BF16, tag="mk")
            mask_a = mk_t[:, :512]
            mask_b = mk_t[:, 512:]
            wvu_t = consts.tile([128, 2, E], BF16, tag="wvu")

            # One contiguous DMA per weight matrix; wk split across both
            # rings so the first KT chain can start as soon as possible.
            wkt = wp.tile([128, 8, D], BF16, tag="wk", name="wkt")
            wvdt = wp.tile([128, 8, D], BF16, tag="wvd", name="wvdt")
            wqt = wp.tile([128, 8, D], BF16, tag="wq", name="wqt")
            nc.sync.dma_start(
                out=wkt[:, :4, :],
                in_=wk[:, : 4 * D].rearrange("p (c d) -> p c d", c=4),
            )
            nc.scalar.dma_start(
                out=wkt[:, 4:, :],
                in_=wk[:, 4 * D :].rearrange("p (c d) -> p c d", c=4),
            )

            # x streams in 1024-query loads (2KB HBM lines), two half-E
            # tiles per load, one per HWDGE ring; each load covers two
            # 512-query processing sub-chunks.
            xtiles = {}

            def load_x(L):
                xa = xs.tile([128, 4, 2 * 512], BF16, tag="xa", bufs=2, name="xa")
                xb = xs.tile([128, 4, 2 * 512], BF16, tag="xb", bufs=2, name="xb")
                xtiles[L] = (xa, xb)
                for t, base, eng in ((xa, 0, nc.sync), (xb, 512, nc.scalar)):
                    if L == 0:
                        # four pieces so the first KT chains start on the
                        # first 256KB
                        for qh in range(2):
                            for ch in range(2):
                                eng.dma_start(
                                    out=t[
                                        :, 2 * ch : 2 * ch + 2, qh * 512 : qh * 512 + 512
                                    ],
                                    in_=xT[
                                        base + ch * 256 : base + ch * 256 + 256,
                                        qh * 512 : qh * 512 + 512,
                                    ].rearrange("(c p) q -> p c q", p=128),
                                )
                            if qh == 0 and base == 0:
                                # weights ordered by first PE use: Vd -> Q
                                nc.sync.dma_start(
                                    out=wvdt,
                                    in_=wvd[:, :].rearrange("p (c d) -> p c d", c=8),
                                )
                            if qh == 0 and base == 512:
                                nc.scalar.dma_start(
                                    out=wqt,
                                    in_=wq[:, :].rearrange("p (c d) -> p c d", c=8),
                                )
                                nc.scalar.dma_start(out=mk_t, in_=mk[:, :])
                    elif L < 3:
                        for qh in range(2):
                            eng.dma_start(
                                out=t[:, :, qh * 512 : qh * 512 + 512],
                                in_=xT[
                                    base : base + 512,
                                    L * 1024 + qh * 512 : L * 1024 + qh * 512 + 512,
                                ].rearrange("(c p) q -> p c q", p=128),
                            )
                    else:
                        eng.dma_start(
                            out=t,
                            in_=xT[
                                base : base + 512, L * 1024 : (L + 1) * 1024
                            ].rearrange("(c p) q -> p c q", p=128),
                        )
                if L == 1:
                    nc.scalar.dma_start(
                        out=wvu_t,
                        in_=wvu[:, :].rearrange("p (c d) -> p c d", c=2),
                    )

            def proj_sub(i):
                """Projections for 512-query sub-chunk i: QT for its
                queries, KT/Vd for the two key blocks embedded in it (KT is
                emitted once per load at N=512, except the piece-split
                first load)."""
                L, s = i // 2, i % 2
                xa, xb = xtiles[L]

                def xch(c):
                    t = xa if c < 4 else xb
                    return t[:, c % 4, s * 512 : (s + 1) * 512]

                # KT[d, keys] from the even column blocks.
                if L == 0 or s == 0:
                    for d in range(2):
                        ps = pps.tile([128, 512], F32, tag="ps")
                        dsl = slice(d * 128, (d + 1) * 128)
                        if L == 0:
                            nkeys, ksl = 256, slice(i * 256, (i + 1) * 256)
                            kview = [_key_view(xch(c)) for c in range(8)]
                        else:
                            nkeys, ksl = 512, slice(L * 512, (L + 1) * 512)
                            kview = [
                                (xa if c < 4 else xb)[:, c % 4, :].rearrange(
                                    "p (g two q) -> p g two q", g=4, two=2, q=128
                                )[:, :, 0, :]
                                for c in range(8)
                            ]
                        for c in range(8):
                            nc.tensor.matmul(
                                ps[:, :nkeys],
                                lhsT=(wkt[:, c, dsl]),
                                rhs=(kview[c]),
                                start=(c == 0),
                                stop=(c == 7),
                            )
                        nc.vector.tensor_copy(kt[d][:, ksl], ps[:, :nkeys])
                # Vd[k, d] (partition = keys) for key blocks 2i, 2i+1, which
                # sit at sub-chunk columns 0:128 and 256:384.
                for h in range(2):
                    pvp = pps.tile([128, 512], F32, tag="ps")
                    csl = slice(h * 256, h * 256 + 128)
                    for c in range(8):
                        nc.tensor.matmul(
                            pvp[:, :D],
                            lhsT=(xch(c)[:, csl]),
                            rhs=(wvdt[:, c, :]),
                            start=(c == 0),
                            stop=(c == 7),
                        )
                    nc.vector.tensor_copy(vd[i][:, h, :], pvp[:, :D])

                for d in range(2):
                    ps = pps.tile([128, 512], F32, tag="ps")
                    dsl = slice(d * 128, (d + 1) * 128)
                    for c in range(8):
                        nc.tensor.matmul(
                            ps,
                            lhsT=(wqt[:, c, dsl]),
                            rhs=(xch(c)),
                            start=(c == 0),
                            stop=(c == 7),
                        )
                    nc.vector.tensor_copy(qt[d][:, i * 512 : (i + 1) * 512], ps)

            def stage_attn(qc, ppool, out_emitters=()):
                """Fused scores + rank-D reduction for query chunk qc.

                Score block kb: ST = K Q^T matmuls + exp + diagonal mask ->
                P tile; the OT'[d, q] += Vd[k, d]^T P[k, q] accumulation for
                block kb trails LAG blocks behind in the PE stream (both
                d-half chains interleaved per block), so each P tile's last
                use follows its exp closely: the P pool needs only 8 ring
                slots, and the PE's per-block cost (3 matmul pairs, ~1.3us)
                exceeds the ACT exp cost (~0.7us), so the exp stream never
                backs the PE up against the 3-deep score-PSUM ring.

                out_emitters: closures emitting the chunk-(qc-1) Wvup
                out-stage, interleaved at blocks 3 and 7 to spread the
                staging-copy and output-DMA load."""
                out_emitters = list(out_emitters)
                nb = 2 * qc + 2
                LAG = 4
                acc = accpool.tile([128, 512], F32, tag="acc")
                otp = [
                    ps_ot.tile([128, 512], F32, tag="otp", name=f"otp{_d}")
                    for _d in range(2)
                ]
                pts = []
                for step in range(nb + LAG):
                  if step < nb:
                    kb = step
                    ksl = slice(kb * 128, (kb + 1) * 128)
                    # The final diagonal block's first 256 query columns are
                    # fully masked for both parities: compute only the upper
                    # half and zero the rest.
                    half = kb == nb - 1
                    csl = slice(256, 512) if half else slice(0, 512)
                    st = pps.tile([128, 512], F32, tag="ps")
                    for d in range(2):
                        nc.tensor.matmul(
                            st[:, csl],
                            lhsT=(kt[d][:, ksl]),
                            rhs=(qt[d][:, qc * 512 + csl.start : qc * 512 + csl.stop]),
                            start=(d == 0),
                            stop=(d == 1),
                        )
                    pt = ppool.tile(
                        [128, 512], BF16, tag=f"p{kb % 4}", bufs=2, name=f"p{kb % 4}"
                    )
                    if half:
                        nc.vector.memset(pt[:, :256], 0.0)
                    nc.scalar.activation(
                        pt[:, csl], st[:, csl],
                        mybir.ActivationFunctionType.Exp, scale=float(SCALE),
                    )
                    # The last two blocks straddle the causal diagonal.
                    if kb == nb - 2:
                        nc.vector.tensor_mul(pt, pt, mask_a)
                    elif kb == nb - 1:
                        nc.vector.tensor_mul(
                            pt[:, 256:], pt[:, 256:], mask_b[:, 256:]
                        )
                    pts.append(pt)
                    # Softmax-denominator accumulation, column-split across
                    # the Pool and DVE engines to keep pace with the exps.
                    if kb == 0:
                        nc.gpsimd.tensor_copy(acc[:, :256], pt[:, :256])
                        nc.vector.tensor_copy(acc[:, 256:], pt[:, 256:])
                    else:
                        nc.gpsimd.tensor_add(acc[:, :256], acc[:, :256], pt[:, :256])
                        nc.vector.tensor_add(acc[:, 256:], acc[:, 256:], pt[:, 256:])
                    if kb in (3, 7) and out_emitters:
                        out_emitters.pop(0)()
                  if step >= LAG:
                    kb = step - LAG
                    for dh in range(2):
                        nc.tensor.matmul(
                            otp[dh],
                            lhsT=(vd[kb // 2][:, kb % 2, dh * 128 : (dh + 1) * 128]),
                            rhs=(pts[kb]),
                            start=(kb == 0),
                            stop=(kb == nb - 1),
                        )
                for em in out_emitters:
                    em()
                return otp, acc

            def stage_reduce(qc, otp, acc, stage):
                """R: stage the rank-D attention result out of PSUM, then
                the softmax denominators."""
                ots = []
                for dh in range(2):
                    ot = stage.tile([128, 512], BF16, tag=f"ot{dh}", bufs=2)
                    # Split the PSUM->SBUF copy so the first Wvup matmuls can
                    # start after half the copy has landed.
                    nc.scalar.activation(
                        ot[:, :256], otp[dh][:, :256],
                        mybir.ActivationFunctionType.Copy,
                    )
                    nc.scalar.activation(
                        ot[:, 256:], otp[dh][:, 256:],
                        mybir.ActivationFunctionType.Copy,
                    )
                    ots.append(ot)
                # sums[1, q] = 1^T acc[k, q] -- a single matmul per chunk; it
                # also covers the ot-copy latency before stage_out's first
                # matmul. The PE wants f32r operands, and f32r matmul inputs
                # must be produced rounded, so DVE stages acc into f32r.
                acc_r = stage.tile([128, 512], F32R, tag="accr", bufs=2)
                nc.scalar.activation(
                    acc_r, acc, mybir.ActivationFunctionType.Copy
                )
                sums = ps_o.tile([128, 512], F32, tag="ops", name="sums")
                nc.tensor.matmul(
                    sums[:1, :], lhsT=(ones_r), rhs=(acc_r), start=True, stop=True
                )
                nc.vector.tensor_copy(srow[:, qc * 512 : (qc + 1) * 512], sums[:1, :])
                return ots

            def make_out_emitters(qc, ots, stage):
                """W: O[q, e] = OT'[d, q]^T Wvup[d, e]; two emitters, one
                per 256-query half, each ending in one output DMA on the
                scalar HWDGE ring. The final chunk instead DMAs each
                128-query block as soon as it is staged (alternating
                rings) to shorten the completion tail."""
                obt = stage.tile([128, 4, E], BF16, tag="obt", bufs=2)
                last = qc == NQC - 1

                def emit(half):
                    for q4 in (2 * half, 2 * half + 1):
                        q4sl = slice(q4 * 128, (q4 + 1) * 128)
                        for eh in range(2):
                            esl = slice(eh * 512, (eh + 1) * 512)
                            ops = ps_o.tile([128, 512], F32, tag="ops")
                            for dh in range(2):
                                nc.tensor.matmul(
                                    ops,
                                    lhsT=(ots[dh][:, q4sl]),
                                    rhs=(wvu_t[:, dh, esl]),
                                    start=(dh == 0),
                                    stop=(dh == 1),
                                )
                            if eh == 0:
                                nc.vector.tensor_copy(obt[:, q4, esl], ops)
                            else:
                                nc.scalar.activation(
                                    obt[:, q4, esl], ops,
                                    mybir.ActivationFunctionType.Copy,
                                )
                        if last:
                            eng = nc.sync if q4 % 2 == 0 else nc.scalar
                            eng.dma_start(
                                out=o[
                                    (qc * 4 + q4) * 128 : (qc * 4 + q4 + 1) * 128, :
                                ],
                                in_=obt[:, q4, :],
                            )
                    if not last:
                        nc.scalar.dma_start(
                            out=o[
                                (qc * 4 + 2 * half) * 128 : (qc * 4 + 2 * half + 2)
                                * 128,
                                :,
                            ].rearrange("(c p) e -> p c e", p=128),
                            in_=obt[:, 2 * half : 2 * half + 2, :],
                        )

                return [lambda: emit(0), lambda: emit(1)]

            # Merged software pipeline: the projection stream (pipe-bound)
            # interleaves with the attention stages (PE/exp-bound) one chunk
            # behind, so each fills the other's stalls. Order per step --
            # proj(i), scores(i-1) [with out(i-2) interleaved], reduce(i-1).
            ots = None
            for i in range(NQC):
                if i == 0:
                    load_x(0)
                    load_x(1)
                elif i in (1, 3):
                    load_x((i + 3) // 2)
                proj_sub(i)
                if i >= 1:
                    ems = make_out_emitters(i - 2, ots, stage) if i >= 2 else ()
                    otp, acc = stage_attn(i - 1, ppool, ems)
                    ots = stage_reduce(i - 1, otp, acc, stage)
            ems = make_out_emitters(NQC - 2, ots, stage)
            otp, acc = stage_attn(NQC - 1, ppool, ems)
            ots = stage_reduce(NQC - 1, otp, acc, stage)
            nc.sync.dma_start(out=ssum[:, :], in_=srow)
            for em in make_out_emitters(NQC - 1, ots, stage):
                em()
    nc.finalize()
    return nc


def _get_nc():
    if "nc" not in _CACHE:
        _CACHE["nc"] = _build_nc()
    return _CACHE["nc"]


def _host_masks(parity: int):
    """Diagonal-block masks in kernel coordinates. For odd cores the query
    order inside each 256-row pair is swapped (host block-pair permutation),
    so the in-chunk query offset is 128*(block^parity) + within."""
    import ml_dtypes

    yb = np.arange(512) // 128
    ym = np.arange(512) % 128
    q_off = 128 * (yb ^ parity) + ym  # [512] original query offset in chunk
    x = np.arange(128)[:, None]
    mask_a = (q_off[None, :] >= 128 * parity + x).astype(ml_dtypes.bfloat16)
    mask_b = (q_off[None, :] >= 256 + 128 * parity + x).astype(ml_dtypes.bfloat16)
    return np.ascontiguousarray(np.concatenate([mask_a, mask_b], axis=1))


def _swap_pairs(rows):
    """Swap adjacent 128-row block pairs along axis 0."""
    n = rows.shape[0]
    return np.ascontiguousarray(
        rows.reshape(n // 256, 2, 128, *rows.shape[1:])[:, ::-1].reshape(rows.shape)
    )


def _pack_w(w):
    """[C*128, F] -> [128, C*F]: row p holds [c, f] for source row c*128+p
    (the kernel's SBUF e-chunk layout, made host-side so the weight DMA is
    one fully contiguous copy)."""
    import ml_dtypes

    w = np.asarray(w, dtype=np.float32).astype(ml_dtypes.bfloat16)
    c, f = w.shape[0] // 128, w.shape[1]
    return np.ascontiguousarray(
        w.reshape(c, 128, f).transpose(1, 0, 2).reshape(128, c * f)
    )


def _make_in_maps(inputs, Wq, Wk, Wvdown, Wvup):
    import ml_dtypes

    inputs = np.asarray(inputs, dtype=np.float32)
    Wq = _pack_w(Wq)
    Wk = _pack_w(Wk)
    Wvdown = _pack_w(Wvdown)
    Wvup = _pack_w(Wvup)

    in_maps = []
    for core in range(NCORES):
        b, parity = core // 2, core % 2
        xb = inputs[b]  # [N, E]
        if parity:
            xb = _swap_pairs(xb)
        xT = np.ascontiguousarray(xb.T).astype(ml_dtypes.bfloat16)  # [E, N]
        in_maps.append(
            {
                "xT": xT,
                "wq": Wq,
                "wk": Wk,
                "wvd": Wvdown,
                "wvu": Wvup,
                "mk": _host_masks(parity),
            }
        )
    return in_maps


def _combine(results):
    out = np.empty((B, N, E), dtype=np.float32)
    for b in range(B):
        o_e = np.asarray(results[2 * b]["o"], dtype=np.float32)
        o_o = np.asarray(results[2 * b + 1]["o"], dtype=np.float32)
        s_e = np.asarray(results[2 * b]["ssum"], dtype=np.float32).reshape(N)
        s_o = np.asarray(results[2 * b + 1]["ssum"], dtype=np.float32).reshape(N)
        # odd-parity core produced rows in block-pair-swapped order
        o_o = _swap_pairs(o_o)
        s_o = _swap_pairs(s_o)
        out[b] = (o_e + o_o) / (s_e + s_o)[:, None]
    return out


def kernel(inputs, Wq, Wk, Wvdown, Wvup):
    from concourse.bass_utils import run_bass_kernel_spmd

    nc = _get_nc()
    in_maps = _make_in_maps(inputs, Wq, Wk, Wvdown, Wvup)
    res = run_bass_kernel_spmd(nc, in_maps, core_ids=list(range(NCORES)))
    return _combine(res.results)


# revision 15
# speedup vs baseline: 1.0286x; 1.0092x over previous
"""Causal dot-product attention (low-rank V) on 8 Trainium2 NeuronCores.

Problem: inputs [B=4, N=4096, E=1024], Wq/Wk/Wvdown [E, D=256], Wvup [D, E].
    Q = x Wq; K = x Wk; S = Q K^T / sqrt(D) (causal); A = softmax(S)
    V = x Wvdown Wvup; out = A V

Sharding: core = (batch, key-parity). Each of the 4 batches is handled by a
pair of cores; core parity c owns the interleaved global key blocks {2j+c}
(128 rows each), which balances the causal work exactly. Each core computes
full Q for its batch, K/Vd for its key half, and produces the *unnormalized*
attention output O_unnorm plus softmax row-sums. The host combines:
out = (O_even + O_odd) / (s_even + s_odd).

The kernel program is parity-uniform: the host swaps adjacent 128-row block
pairs of x for odd cores (and hands matching diagonal masks), so every core's
keys sit at the EVEN 128-column blocks of its query stream. K and Vd then
project strided SBUF views of the already-loaded x tiles -- no separate
key-activation DMA at all. The host un-swaps the odd cores' output rows.

Low-rank V is exploited on-device: O = A V = (A (x Wvd)) Wvup, so the wide
(E=1024) contraction happens once per query row against the rank-D attention
result instead of once per (query, key-block) pair. Scores are computed
transposed, ST[k, q] = K Q^T, so the exp'd tile P[k, q] is directly the
moving operand of the OT' = Vd^T P accumulation (no on-device transposes).

All activations/weights stream as bf16 (error budget ~0.5% vs the 2e-2
gate): halves HBM traffic and enables FWL fast weight loads on the PE.
A short dummy-matmul chain at kernel start keeps the PE busy through the
initial DMA wait so the HAM clock gate reaches 2.4 GHz before real work.
The scores stage is interleaved with the previous chunk's Wvup out-stage so
the ACT engine's exp stream (690ns/tile vs the PE's 426ns/tile) never
backs the PE up against the 3-deep score-PSUM ring.
"""

import sys

sys.path.insert(0, "/opt/trn_rl_repo")

import numpy as np

import concourse.bacc as bacc
import concourse.mybir as mybir
import concourse.tile as tile

F32 = mybir.dt.float32
F32R = mybir.dt.float32r
BF16 = mybir.dt.bfloat16

B, N, E, D = 4, 4096, 1024, 256
NCORES = 8
KLOC = N // 2  # local keys per core
NKB = KLOC // 128  # 16 local key blocks
NQC = N // 512  # 8 query chunks of 512
SCALE = 1.0 / np.sqrt(np.float32(D))  # 1/16

_CACHE = {}


def _key_view(x_ap):
    """Strided view of a [128, 512] x-chunk AP selecting its two even
    128-column blocks (the key columns) as a [128, 2, 128] AP."""
    return x_ap.rearrange("p (g two q) -> p g two q", g=2, two=2, q=128)[:, :, 0, :]


def _build_nc(reps=1):
    nc = bacc.Bacc("TRN2", target_bir_lowering=False)

    # Weights arrive host-packed in the SBUF e-chunk layout ([128, 8, 256]
    # flattened: row p holds [c, d] for e-row c*128+p) so each is one fully
    # contiguous DMA.
    xT = nc.dram_tensor("xT", [E, N], BF16, kind="ExternalInput")
    wq = nc.dram_tensor("wq", [128, E * D // 128], BF16, kind="ExternalInput")
    wk = nc.dram_tensor("wk", [128, E * D // 128], BF16, kind="ExternalInput")
    wvd = nc.dram_tensor("wvd", [128, E * D // 128], BF16, kind="ExternalInput")
    wvu = nc.dram_tensor("wvu", [128, D * E // 128], BF16, kind="ExternalInput")
    mk = nc.dram_tensor("mk", [128, 1024], BF16, kind="ExternalInput")

    o = nc.dram_tensor("o", [N, E], BF16, kind="ExternalOutput")
    ssum = nc.dram_tensor("ssum", [1, N], F32, kind="ExternalOutput")

    with tile.TileContext(nc) as tc:
      for _rep in range(reps):
        with (
            tc.tile_pool(name=f"res{_rep}", bufs=1) as res,
            tc.tile_pool(name=f"consts{_rep}", bufs=1) as consts,
            tc.tile_pool(name=f"wpool{_rep}", bufs=1) as wp,
            tc.tile_pool(name=f"xstream{_rep}", bufs=3) as xs,
            tc.tile_pool(name=f"accpool{_rep}", bufs=2) as accpool,
            tc.tile_pool(name=f"ppool{_rep}", bufs=2) as ppool,
            tc.tile_pool(name=f"stage{_rep}", bufs=3) as stage,
            tc.tile_pool(name=f"ps{_rep}", bufs=4, space="PSUM") as pps,
            tc.tile_pool(name=f"ps_ot{_rep}", bufs=2, space="PSUM") as ps_ot,
            tc.tile_pool(name=f"ps_o{_rep}", bufs=2, space="PSUM") as ps_o,
        ):
            # PE warm-up: a dependency-free accumulation chain issued ahead
            # of everything keeps the PE busy through the initial DMA wait
            # (~7us: weights + first x pieces at ~130 GB/s/ring), so the HAM
            # clock gate un-throttles before the first real matmul and the
            # activity window stays hot through the DMA-paced first chunk.
            wrm = consts.tile([128, 128], BF16, tag="wrm")
            nc.gpsimd.memset(wrm, 0.0)
            scr = consts.tile([1, 1], F32, tag="scr")
            wps = pps.tile([128, 512], F32, tag="ps")
            NWARM = 48
            for j in range(NWARM):
                nc.tensor.matmul(
                    wps[:, :128], lhsT=wrm, rhs=wrm,
                    start=(j == 0), stop=(j == NWARM - 1),
                )
            nc.vector.tensor_copy(scr, wps[:1, :1])

            # Resident results of the projection phase.
            qt = [res.tile([128, N], BF16, tag=f"qt{d}", name=f"qt{d}") for d in range(2)]
            kt = [res.tile([128, KLOC], BF16, tag=f"kt{d}", name=f"kt{d}") for d in range(2)]
            # Vd tiles grouped per chunk (2 key blocks each): block kb lives
            # at vd[kb // 2][:, kb % 2, :].
            vd = [
                res.tile([128, 2, D], BF16, tag=f"vd{i}", name=f"vd{i}")
                for i in range(NQC)
            ]
            srow = res.tile([1, N], F32, tag="srow")

            ones_f = consts.tile([128, 1], F32, tag="ones_f")
            nc.vector.memset(ones_f, 1.0)
            ones_r = consts.tile([128, 1], F32R, tag="ones_r")
            nc.vector.tensor_copy(ones_r, ones_f)
            mk_t = consts.tile([128, 1024], BF16, tag="mk")
            mask_a = mk_t[:, :512]
            mask_b = mk_t[:, 512:]
            wvu_t = consts.tile([128, 2, E], BF16, tag="wvu")

            # One contiguous DMA per weight matrix; wk split across both
            # rings so the first KT chain can start as soon as possible.
            wkt = wp.tile([128, 8, D], BF16, tag="wk", name="wkt")
            wvdt = wp.tile([128, 8, D], BF16, tag="wvd", name="wvdt")
            wqt = wp.tile([128, 8, D], BF16, tag="wq", name="wqt")
            nc.sync.dma_start(
                out=wkt[:, :4, :],
                in_=wk[:, : 4 * D].rearrange("p (c d) -> p c d", c=4),
            )
            nc.scalar.dma_start(
                out=wkt[:, 4:, :],
                in_=wk[:, 4 * D :].rearrange("p (c d) -> p c d", c=4),
            )

            # x streams in 1024-query loads (2KB HBM lines), two half-E
            # tiles per load, one per HWDGE ring; each load covers two
            # 512-query processing sub-chunks.
            xtiles = {}

            def load_x(L):
                xa = xs.tile([128, 4, 2 * 512], BF16, tag="xa", bufs=2, name="xa")
                xb = xs.tile([128, 4, 2 * 512], BF16, tag="xb", bufs=2, name="xb")
                xtiles[L] = (xa, xb)
                for t, base, eng in ((xa, 0, nc.sync), (xb, 512, nc.scalar)):
                    if L == 0:
                        # four pieces so the first KT chains start on the
                        # first 256KB
                        for qh in range(2):
                            for ch in range(2):
                                eng.dma_start(
                                    out=t[
                                        :, 2 * ch : 2 * ch + 2, qh * 512 : qh * 512 + 512
                                    ],
                                    in_=xT[
                                        base + ch * 256 : base + ch * 256 + 256,
                                        qh * 512 : qh * 512 + 512,
                                    ].rearrange("(c p) q -> p c q", p=128),
                                )
                            if qh == 0 and base == 0:
                                # weights ordered by first PE use: Vd -> Q
                                nc.sync.dma_start(
                                    out=wvdt,
                                    in_=wvd[:, :].rearrange("p (c d) -> p c d", c=8),
                                )
                            if qh == 0 and base == 512:
                                nc.scalar.dma_start(
                                    out=wqt,
                                    in_=wq[:, :].rearrange("p (c d) -> p c d", c=8),
                                )
                                nc.scalar.dma_start(out=mk_t, in_=mk[:, :])
                    elif L < 3:
                        for qh in range(2):
                            eng.dma_start(
                                out=t[:, :, qh * 512 : qh * 512 + 512],
                                in_=xT[
                                    base : base + 512,
                                    L * 1024 + qh * 512 : L * 1024 + qh * 512 + 512,
                                ].rearrange("(c p) q -> p c q", p=128),
                            )
                    else:
                        eng.dma_start(
                            out=t,
                            in_=xT[
                                base : base + 512, L * 1024 : (L + 1) * 1024
                            ].rearrange("(c p) q -> p c q", p=128),
                        )
                if L == 1:
                    nc.scalar.dma_start(
                        out=wvu_t,
                        in_=wvu[:, :].rearrange("p (c d) -> p c d", c=2),
                    )

            def warm_fill(n):
                """Dummy PE chain: fills DMA-paced stalls in the head so
                the HAM activity window stays hot."""
                fps = pps.tile([128, 512], F32, tag="ps", name="fps")
                for j in range(n):
                    nc.tensor.matmul(
                        fps[:, :128], lhsT=wrm, rhs=wrm,
                        start=(j == 0), stop=(j == n - 1),
                    )
                nc.vector.tensor_copy(scr, fps[:1, :1])

            def proj_sub(i):
                """Projections for 512-query sub-chunk i: QT for its
                queries, KT/Vd for the two key blocks embedded in it (KT is
                emitted once per load at N=512, except the piece-split
                first load)."""
                L, s = i // 2, i % 2
                xa, xb = xtiles[L]

                def xch(c):
                    t = xa if c < 4 else xb
                    return t[:, c % 4, s * 512 : (s + 1) * 512]

                # KT[d, keys] from the even column blocks.
                if L == 0 or s == 0:
                    for d in range(2):
                        ps = pps.tile([128, 512], F32, tag="ps")
                        dsl = slice(d * 128, (d + 1) * 128)
                        if L == 0:
                            nkeys, ksl = 256, slice(i * 256, (i + 1) * 256)
                            kview = [_key_view(xch(c)) for c in range(8)]
                        else:
                            nkeys, ksl = 512, slice(L * 512, (L + 1) * 512)
                            kview = [
                                (xa if c < 4 else xb)[:, c % 4, :].rearrange(
                                    "p (g two q) -> p g two q", g=4, two=2, q=128
                                )[:, :, 0, :]
                                for c in range(8)
                            ]
                        for c in range(8):
                            nc.tensor.matmul(
                                ps[:, :nkeys],
                                lhsT=(wkt[:, c, dsl]),
                                rhs=(kview[c]),
                                start=(c == 0),
                                stop=(c == 7),
                            )
                        nc.vector.tensor_copy(kt[d][:, ksl], ps[:, :nkeys])
                # Vd[k, d] (partition = keys) for key blocks 2i, 2i+1, which
                # sit at sub-chunk columns 0:128 and 256:384.
                for h in range(2):
                    pvp = pps.tile([128, 512], F32, tag="ps")
                    csl = slice(h * 256, h * 256 + 128)
                    for c in range(8):
                        nc.tensor.matmul(
                            pvp[:, :D],
                            lhsT=(xch(c)[:, csl]),
                            rhs=(wvdt[:, c, :]),
                            start=(c == 0),
                            stop=(c == 7),
                        )
                    nc.vector.tensor_copy(vd[i][:, h, :], pvp[:, :D])
                if i <= 1:
                    warm_fill(8)

                for d in range(2):
                    ps = pps.tile([128, 512], F32, tag="ps")
                    dsl = slice(d * 128, (d + 1) * 128)
                    for c in range(8):
                        nc.tensor.matmul(
                            ps,
                            lhsT=(wqt[:, c, dsl]),
                            rhs=(xch(c)),
                            start=(c == 0),
                            stop=(c == 7),
                        )
                    nc.vector.tensor_copy(qt[d][:, i * 512 : (i + 1) * 512], ps)
                if i <= 1:
                    warm_fill(8)

            def stage_attn(qc, ppool, out_emitters=()):
                """Fused scores + rank-D reduction for query chunk qc.

                Score block kb: ST = K Q^T matmuls + exp + diagonal mask ->
                P tile; the OT'[d, q] += Vd[k, d]^T P[k, q] accumulation for
                block kb trails LAG blocks behind in the PE stream (both
                d-half chains interleaved per block), so each P tile's last
                use follows its exp closely: the P pool needs only 8 ring
                slots, and the PE's per-block cost (3 matmul pairs, ~1.3us)
                exceeds the ACT exp cost (~0.7us), so the exp stream never
                backs the PE up against the 3-deep score-PSUM ring.

                out_emitters: closures emitting the chunk-(qc-1) Wvup
                out-stage, interleaved at blocks 3 and 7 to spread the
                staging-copy and output-DMA load."""
                out_emitters = list(out_emitters)
                nb = 2 * qc + 2
                LAG = 4
                acc = accpool.tile([128, 512], F32, tag="acc")
                otp = [
                    ps_ot.tile([128, 512], F32, tag="otp", name=f"otp{_d}")
                    for _d in range(2)
                ]
                pts = []
                for step in range(nb + LAG):
                  if step < nb:
                    kb = step
                    ksl = slice(kb * 128, (kb + 1) * 128)
                    # The final diagonal block's first 256 query columns are
                    # fully masked for both parities: compute only the upper
                    # half and zero the rest.
                    half = kb == nb - 1
                    csl = slice(256, 512) if half else slice(0, 512)
                    st = pps.tile([128, 512], F32, tag="ps")
                    for d in range(2):
                        nc.tensor.matmul(
                            st[:, csl],
                            lhsT=(kt[d][:, ksl]),
                            rhs=(qt[d][:, qc * 512 + csl.start : qc * 512 + csl.stop]),
                            start=(d == 0),
                            stop=(d == 1),
                        )
                    pt = ppool.tile(
                        [128, 512], BF16, tag=f"p{kb % 4}", bufs=2, name=f"p{kb % 4}"
                    )
                    if half:
                        nc.vector.memset(pt[:, :256], 0.0)
                    nc.scalar.activation(
                        pt[:, csl], st[:, csl],
                        mybir.ActivationFunctionType.Exp, scale=float(SCALE),
                    )
                    # The last two blocks straddle the causal diagonal.
                    if kb == nb - 2:
                        nc.vector.tensor_mul(pt, pt, mask_a)
                    elif kb == nb - 1:
                        nc.vector.tensor_mul(
                            pt[:, 256:], pt[:, 256:], mask_b[:, 256:]
                        )
                    pts.append(pt)
                    # Softmax-denominator accumulation, column-split across
                    # the Pool and DVE engines to keep pace with the exps.
                    if kb == 0:
                        nc.gpsimd.tensor_copy(acc[:, :256], pt[:, :256])
                        nc.vector.tensor_copy(acc[:, 256:], pt[:, 256:])
                    else:
                        nc.gpsimd.tensor_add(acc[:, :256], acc[:, :256], pt[:, :256])
                        nc.vector.tensor_add(acc[:, 256:], acc[:, 256:], pt[:, 256:])
                    if kb in (3, 7) and out_emitters:
                        out_emitters.pop(0)()
                  if step >= LAG:
                    kb = step - LAG
                    for dh in range(2):
                        nc.tensor.matmul(
                            otp[dh],
                            lhsT=(vd[kb // 2][:, kb % 2, dh * 128 : (dh + 1) * 128]),
                            rhs=(pts[kb]),
                            start=(kb == 0),
                            stop=(kb == nb - 1),
                        )
                for em in out_emitters:
                    em()
                return otp, acc

            def emit_sums(qc, acc):
                """sums[1, q] = 1^T acc[k, q] -- a single matmul per chunk.
                The PE wants f32r operands, and f32r matmul inputs must be
                produced rounded, so ACT stages acc into f32r."""
                acc_r = stage.tile([128, 512], F32R, tag="accr", bufs=2)
                nc.scalar.activation(
                    acc_r, acc, mybir.ActivationFunctionType.Copy
                )
                sums = ps_o.tile([128, 512], F32, tag="ops", name="sums")
                nc.tensor.matmul(
                    sums[:1, :], lhsT=(ones_r), rhs=(acc_r), start=True, stop=True
                )
                nc.vector.tensor_copy(srow[:, qc * 512 : (qc + 1) * 512], sums[:1, :])

            def stage_reduce(qc, otp, acc, stage, defer_sums=False):
                """R: stage the rank-D attention result out of PSUM
                (half-major, so the first Wvup matmuls unblock after two
                copies), then the softmax denominators."""
                ots = [
                    stage.tile([128, 512], BF16, tag=f"ot{dh}", bufs=2, name=f"ot{dh}")
                    for dh in range(2)
                ]
                for csl in (slice(0, 256), slice(256, 512)):
                    for dh in range(2):
                        nc.scalar.activation(
                            ots[dh][:, csl], otp[dh][:, csl],
                            mybir.ActivationFunctionType.Copy,
                        )
                if not defer_sums:
                    # mid-pipeline: the sums matmul also covers the ot-copy
                    # latency before the next out-stage's first matmul
                    emit_sums(qc, acc)
                return ots

            def make_out_emitters(qc, ots, stage):
                """W: O[q, e] = OT'[d, q]^T Wvup[d, e]; two emitters, one
                per 256-query half, each ending in one output DMA on the
                scalar HWDGE ring. The final chunk instead DMAs each
                128-query block as soon as it is staged (alternating
                rings) to shorten the completion tail."""
                obt = stage.tile([128, 4, E], BF16, tag="obt", bufs=2)
                last = qc == NQC - 1

                def emit(half):
                    for q4 in (2 * half, 2 * half + 1):
                        q4sl = slice(q4 * 128, (q4 + 1) * 128)
                        for eh in range(2):
                            esl = slice(eh * 512, (eh + 1) * 512)
                            ops = ps_o.tile([128, 512], F32, tag="ops")
                            for dh in range(2):
                                nc.tensor.matmul(
                                    ops,
                                    lhsT=(ots[dh][:, q4sl]),
                                    rhs=(wvu_t[:, dh, esl]),
                                    start=(dh == 0),
                                    stop=(dh == 1),
                                )
                            if eh == 0:
                                nc.vector.tensor_copy(obt[:, q4, esl], ops)
                            else:
                                nc.scalar.activation(
                                    obt[:, q4, esl], ops,
                                    mybir.ActivationFunctionType.Copy,
                                )
                        if last:
                            eng = nc.sync if q4 % 2 == 0 else nc.scalar
                            eng.dma_start(
                                out=o[
                                    (qc * 4 + q4) * 128 : (qc * 4 + q4 + 1) * 128, :
                                ],
                                in_=obt[:, q4, :],
                            )
                    if not last:
                        nc.scalar.dma_start(
                            out=o[
                                (qc * 4 + 2 * half) * 128 : (qc * 4 + 2 * half + 2)
                                * 128,
                                :,
                            ].rearrange("(c p) e -> p c e", p=128),
                            in_=obt[:, 2 * half : 2 * half + 2, :],
                        )

                return [lambda: emit(0), lambda: emit(1)]

            # Merged software pipeline: the projection stream (pipe-bound)
            # interleaves with the attention stages (PE/exp-bound) one chunk
            # behind, so each fills the other's stalls. Order per step --
            # proj(i), scores(i-1) [with out(i-2) interleaved], reduce(i-1).
            ots = None
            for i in range(NQC):
                if i == 0:
                    load_x(0)
                    load_x(1)
                elif i in (1, 3):
                    load_x((i + 3) // 2)
                proj_sub(i)
                if i >= 1:
                    ems = make_out_emitters(i - 2, ots, stage) if i >= 2 else ()
                    otp, acc = stage_attn(i - 1, ppool, ems)
                    ots = stage_reduce(i - 1, otp, acc, stage)
            ems = make_out_emitters(NQC - 2, ots, stage)
            otp, acc = stage_attn(NQC - 1, ppool, ems)
            ots = stage_reduce(NQC - 1, otp, acc, stage, defer_sums=True)
            for em in make_out_emitters(NQC - 1, ots, stage):
                em()
            emit_sums(NQC - 1, acc)
            nc.sync.dma_start(out=ssum[:, :], in_=srow)
    nc.finalize()
    return nc


def _get_nc():
    if "nc" not in _CACHE:
        _CACHE["nc"] = _build_nc()
    return _CACHE["nc"]


def _host_masks(parity: int):
    """Diagonal-block masks in kernel coordinates. For odd cores the query
    order inside each 256-row pair is swapped (host block-pair permutation),
    so the in-chunk query offset is 128*(block^parity) + within."""
    import ml_dtypes

    yb = np.arange(512) // 128
    ym = np.arange(512) % 128
    q_off = 128 * (yb ^ parity) + ym  # [512] original query offset in chunk
    x = np.arange(128)[:, None]
    mask_a = (q_off[None, :] >= 128 * parity + x).astype(ml_dtypes.bfloat16)
    mask_b = (q_off[None, :] >= 256 + 128 * parity + x).astype(ml_dtypes.bfloat16)
    return np.ascontiguousarray(np.concatenate([mask_a, mask_b], axis=1))


def _swap_pairs(rows):
    """Swap adjacent 128-row block pairs along axis 0."""
    n = rows.shape[0]
    return np.ascontiguousarray(
        rows.reshape(n // 256, 2, 128, *rows.shape[1:])[:, ::-1].reshape(rows.shape)
    )


def _pack_w(w):
    """[C*128, F] -> [128, C*F]: row p holds [c, f] for source row c*128+p
    (the kernel's SBUF e-chunk layout, made host-side so the weight DMA is
    one fully contiguous copy)."""
    import ml_dtypes

    w = np.asarray(w, dtype=np.float32).astype(ml_dtypes.bfloat16)
    c, f = w.shape[0] // 128, w.shape[1]
    return np.ascontiguousarray(
        w.reshape(c, 128, f).transpose(1, 0, 2).reshape(128, c * f)
    )


def _make_in_maps(inputs, Wq, Wk, Wvdown, Wvup):
    import ml_dtypes

    inputs = np.asarray(inputs, dtype=np.float32)
    Wq = _pack_w(Wq)
    Wk = _pack_w(Wk)
    Wvdown = _pack_w(Wvdown)
    Wvup = _pack_w(Wvup)

    in_maps = []
    for core in range(NCORES):
        b, parity = core // 2, core % 2
        xb = inputs[b]  # [N, E]
        if parity:
            xb = _swap_pairs(xb)
        xT = np.ascontiguousarray(xb.T).astype(ml_dtypes.bfloat16)  # [E, N]
        in_maps.append(
            {
                "xT": xT,
                "wq": Wq,
                "wk": Wk,
                "wvd": Wvdown,
                "wvu": Wvup,
                "mk": _host_masks(parity),
            }
        )
    return in_maps


def _combine(results):
    out = np.empty((B, N, E), dtype=np.float32)
    for b in range(B):
        o_e = np.asarray(results[2 * b]["o"], dtype=np.float32)
        o_o = np.asarray(results[2 * b + 1]["o"], dtype=np.float32)
        s_e = np.asarray(results[2 * b]["ssum"], dtype=np.float32).reshape(N)
        s_o = np.asarray(results[2 * b + 1]["ssum"], dtype=np.float32).reshape(N)
        # odd-parity core produced rows in block-pair-swapped order
        o_o = _swap_pairs(o_o)
        s_o = _swap_pairs(s_o)
        out[b] = (o_e + o_o) / (s_e + s_o)[:, None]
    return out


def kernel(inputs, Wq, Wk, Wvdown, Wvup):
    from concourse.bass_utils import run_bass_kernel_spmd

    nc = _get_nc()
    in_maps = _make_in_maps(inputs, Wq, Wk, Wvdown, Wvup)
    res = run_bass_kernel_spmd(nc, in_maps, core_ids=list(range(NCORES)))
    return _combine(res.results)
